# revision 8
# baseline (speedup 1.0000x reference)
"""Trainium2 Bass kernel for variable-window left/right max pooling.

out[b, c, t] = max(feat[b, c, max(t-L,0) : t+1]) + max(feat[b, c, t : min(t+R,T)])
with L = max(0, round(reg[b,t,0])), R = clip(round(reg[b,t,1]), 1, T).

Strategy (2 batches/core, pure data parallel over 8 cores, fp16 on device):
  - sparse table built directly in t-major layout [t%128, lev, t//128, c]:
    level k = max(level k-1, level k-1 shifted by 2^(k-1) tokens). The token
    shift crosses partitions, which compute engines cannot do (quadrant
    rule), so the shifted copy is produced by a 3-piece SBUF->SBUF DMA
    "rotate" (DMA partition access is unrestricted), then a single in-place
    DVE max per level.
  - each level is stored to a per-batch DRAM table [NLEV*T, C] (t-major
    rows) as soon as it is built.
  - the 4 RMQ terms per token (left a/b, right a/b) are fetched with 16
    hardware indirect-DMA row gathers (offsets [128, 1] int32, one row per
    partition), landing t-major in SBUF.
  - combine on DVE: max(la, lb) + max(ra, rb); store out t-major; host
    transposes [B, T, C] -> [B, C, T].
"""

import sys
import types

import numpy as np


def _install_profile_shim():
    if "antenv.axon_hooks" in sys.modules:
        return
    try:
        hooks = types.ModuleType("antenv.axon_hooks")
        hooks._hook = None
        hooks.set_axon_ntff_profile_hook = lambda h: setattr(hooks, "_hook", h)
        hooks.get_axon_ntff_profile_hook = lambda: hooks._hook
        sys.modules["antenv.axon_hooks"] = hooks
        import antenv

        antenv.axon_hooks = hooks
        from trn_agent_boot.trn_boot import _ntff_profile_via_ctypes

        hooks.set_axon_ntff_profile_hook(
            _ntff_profile_via_ctypes("/opt/axon/libaxon_pjrt.so")
        )
    except Exception:
        pass


_install_profile_shim()

import concourse.bacc as bacc
import concourse.bass as bass
import concourse.mybir as mybir
from concourse.bass_utils import run_bass_kernel_spmd

B, C, T = 16, 1024, 256
N_CORES = 8
BPC = B // N_CORES
NLEV = 6  # sparse-table levels 0..5 (windows up to 33)
NG = 8    # gathers per batch: 4 terms x 2 token chunks
TPAD = 32  # dram table pad rows (rotate loads may overrun logically)

_LOG2 = np.zeros(65, dtype=np.int64)
for _n in range(1, 65):
    _LOG2[_n] = _n.bit_length() - 1

_CACHE = {}
LAST_RESULT = None


def _build_graph():
    if "nc" in _CACHE:
        return _CACHE["nc"]

    nc = bacc.Bacc("TRN2", target_bir_lowering=False, debug=False,
                   num_devices=N_CORES)
    f16 = mybir.dt.float16
    i32 = mybir.dt.int32

    featT_ext = nc.dram_tensor("featT", [BPC, T, C], f16,
                               kind="ExternalInput").ap()
    offs_ext = nc.dram_tensor("offs", [128, BPC * NG], i32,
                              kind="ExternalInput").ap()
    out_ext = nc.dram_tensor("out", [BPC, T, C], f16,
                             kind="ExternalOutput").ap()

    tbl = [nc.dram_tensor(f"tbl{b}", [NLEV * T + TPAD, C], f16).ap()
           for b in range(BPC)]

    ttab = [nc.alloc_sbuf_tensor(f"ttab{b}", [128, NLEV, 2, C], f16).ap()
            for b in range(BPC)]
    gout = [nc.alloc_sbuf_tensor(f"gout{b}", [128, 4, 2, C], f16).ap()
            for b in range(BPC)]
    obuf = [nc.alloc_sbuf_tensor(f"obuf{b}", [128, 2, C], f16).ap()
            for b in range(BPC)]
    offs_sb = nc.alloc_sbuf_tensor("offs_sb", [128, BPC * NG], i32).ap()

    with nc.Block() as block:
        s_in = [nc.alloc_semaphore(f"s_in{b}") for b in range(BPC)]
        s_ino = nc.alloc_semaphore("s_ino")
        s_rot = [nc.alloc_semaphore(f"s_rot{b}") for b in range(BPC)]
        s_lvl = [nc.alloc_semaphore(f"s_lvl{b}") for b in range(BPC)]
        s_st = [nc.alloc_semaphore(f"s_st{b}") for b in range(BPC)]
        s_g = [nc.alloc_semaphore(f"s_g{b}") for b in range(BPC)]
        s_cmb = [nc.alloc_semaphore(f"s_cmb{b}") for b in range(BPC)]
        s_out = [nc.alloc_semaphore(f"s_out{b}") for b in range(BPC)]

        def emit_build(eng, b):
            # Shifted copy of level k-1 into level k's slab (big piece; the
            # tiny partition-wrap pieces run on gpsimd), then the per-level
            # table store. Store k is issued after rotate k+1 so its queue
            # post doesn't delay the build chain.
            for k in range(1, NLEV):
                s = 1 << (k - 1)
                if k == 1:
                    eng.wait_ge(s_in[b], 16)
                else:
                    eng.wait_ge(s_lvl[b], k - 1)
                eng.dma_start(
                    out=ttab[b][0:128 - s, k, :, :],
                    in_=ttab[b][s:128, k - 1, :, :],
                ).then_inc(s_rot[b], 16)
                sk = k - 1
                eng.dma_start(
                    out=tbl[b][sk * T:(sk + 1) * T].rearrange(
                        "(tt p) c -> p tt c", p=128),
                    in_=ttab[b][:, sk, :, :],
                ).then_inc(s_st[b], 16)
            eng.wait_ge(s_lvl[b], NLEV - 1)
            sk = NLEV - 1
            eng.dma_start(
                out=tbl[b][sk * T:(sk + 1) * T].rearrange(
                    "(tt p) c -> p tt c", p=128),
                in_=ttab[b][:, sk, :, :],
            ).then_inc(s_st[b], 16)

        @block.sync
        def _(sync):
            for b in range(BPC):
                sync.dma_start(
                    out=ttab[b][:, 0, :, :],
                    in_=featT_ext[b].rearrange("(tt p) c -> p tt c", p=128),
                ).then_inc(s_in[b], 16)
            sync.dma_start(out=offs_sb, in_=offs_ext).then_inc(s_ino, 16)
            emit_build(sync, 1)
            for b in range(BPC):
                sync.wait_ge(s_cmb[b], 3)
                sync.dma_start(
                    out=out_ext[b].rearrange("(tt p) c -> p tt c", p=128),
                    in_=obuf[b][:, :, :],
                ).then_inc(s_out[b], 16)
            for b in range(BPC):
                sync.wait_ge(s_out[b], 16)

        @block.scalar
        def _(scalar):
            emit_build(scalar, 0)

        @block.vector
        def _(vector):
            for k in range(1, NLEV):
                for b in range(BPC):
                    vector.wait_ge(s_rot[b], 48 * k)
                    vector.tensor_tensor(
                        out=ttab[b][:, k, :, :],
                        in0=ttab[b][:, k, :, :],
                        in1=ttab[b][:, k - 1, :, :],
                        op=mybir.AluOpType.max,
                    ).then_inc(s_lvl[b], 1)
            for b in range(BPC):
                vector.wait_ge(s_g[b], 64)
                vector.tensor_tensor(
                    out=gout[b][:, 0, :, :],
                    in0=gout[b][:, 0, :, :],
                    in1=gout[b][:, 1, :, :],
                    op=mybir.AluOpType.max,
                ).then_inc(s_cmb[b], 1)
                vector.wait_ge(s_g[b], 128)
                vector.tensor_tensor(
                    out=gout[b][:, 2, :, :],
                    in0=gout[b][:, 2, :, :],
                    in1=gout[b][:, 3, :, :],
                    op=mybir.AluOpType.max,
                ).then_inc(s_cmb[b], 1)
                vector.tensor_tensor(
                    out=obuf[b][:, :, :],
                    in0=gout[b][:, 0, :, :],
                    in1=gout[b][:, 2, :, :],
                    op=mybir.AluOpType.add,
                ).then_inc(s_cmb[b], 1)

        @block.gpsimd
        def _(gpsimd):
            # partition-wrap rotate pieces (tiny; SWDGE descriptor count is
            # s <= 16 rows per piece)
            for k in range(1, NLEV):
                s = 1 << (k - 1)
                for b in range(BPC):
                    if k == 1:
                        gpsimd.wait_ge(s_in[b], 16)
                    else:
                        gpsimd.wait_ge(s_lvl[b], k - 1)
                    gpsimd.dma_start(
                        out=ttab[b][128 - s:128, k, 0, :],
                        in_=ttab[b][0:s, k - 1, 1, :],
                    ).then_inc(s_rot[b], 16)
                    gpsimd.dma_start(
                        out=ttab[b][128 - s:128, k, 1, :],
                        in_=ttab[b][128 - s:128, k - 1, 1, :],
                    ).then_inc(s_rot[b], 16)
            gpsimd.wait_ge(s_ino, 16)
            for b in range(BPC):
                gpsimd.wait_ge(s_st[b], 96)
                for g in range(NG):
                    gpsimd.indirect_dma_start(
                        out=gout[b][:, g // 2, g % 2, :],
                        out_offset=None,
                        in_=tbl[b],
                        in_offset=bass.IndirectOffsetOnAxis(
                            ap=offs_sb[:, b * NG + g:b * NG + g + 1], axis=0),
                    ).then_inc(s_g[b], 16)

    nc.compile()
    _CACHE["nc"] = nc
    return nc


def _host_rows(reg):
    """Table row indices [B, 4, T] for terms (la, lb, ra, rb);
    row(level, x) = level * T + x."""
    t = np.arange(T, dtype=np.int64)[None, :]

    rl = np.maximum(np.round(reg[:, :, 0]).astype(np.int64), 0)
    l_left = np.maximum(t - rl, 0)
    len_l = t + 1 - l_left
    k_l = np.where(len_l <= 64, _LOG2[np.minimum(len_l, 64)],
                   np.floor(np.log2(len_l)).astype(np.int64))
    p_l = (1 << k_l).astype(np.int64)
    la = k_l * T + l_left
    lb = k_l * T + (t + 1 - p_l)

    rr = np.clip(np.round(reg[:, :, 1]).astype(np.int64), 1, T)
    r_right = np.minimum(t + rr, T)
    len_r = r_right - t
    k_r = np.where(len_r <= 64, _LOG2[np.minimum(len_r, 64)],
                   np.floor(np.log2(len_r)).astype(np.int64))
    p_r = (1 << k_r).astype(np.int64)
    ra = k_r * T + (t + np.zeros_like(rr))
    rb = k_r * T + (r_right - p_r)

    rows = np.stack([la, lb, ra, rb], axis=1)  # [B, 4, T]
    assert rows.min() >= 0 and rows.max() < NLEV * T, (rows.min(), rows.max())
    return rows


def kernel(feat: np.ndarray, reg: np.ndarray) -> np.ndarray:
    global LAST_RESULT
    feat = np.ascontiguousarray(feat, dtype=np.float32)
    reg = np.ascontiguousarray(reg, dtype=np.float32)
    assert feat.shape == (B, C, T) and reg.shape == (B, T, 2)

    feat16 = feat.astype(np.float16)
    featT = np.ascontiguousarray(feat16.transpose(0, 2, 1))  # [B, T, C]
    rows = _host_rows(reg)  # [B, 4, T]
    # offs[b][p, g] with g = term*2 + tt covering token t = tt*128 + p;
    # uploaded as [128, BPC*NG] per core (partition-major, contiguous).
    offs = rows.reshape(B, 4, 2, 128).reshape(B, 8, 128).astype(np.int32)
    offs = np.ascontiguousarray(offs.transpose(0, 2, 1))  # [B, 128, 8]

    nc = _build_graph()
    in_maps = []
    for i in range(N_CORES):
        sl = slice(i * BPC, (i + 1) * BPC)
        in_maps.append({
            "featT": np.ascontiguousarray(featT[sl]),
            "offs": np.ascontiguousarray(
                offs[sl].transpose(1, 0, 2).reshape(128, BPC * NG)),
        })

    res = run_bass_kernel_spmd(nc, in_maps, list(range(N_CORES)))
    LAST_RESULT = res
    outT = np.concatenate([res.results[i]["out"] for i in range(N_CORES)],
                          axis=0)  # [B, T, C] fp16
    return np.ascontiguousarray(outT.transpose(0, 2, 1)).astype(np.float32)


# revision 15
# speedup vs baseline: 1.7325x; 1.7325x over previous
"""Trainium2 Bass kernel for variable-window left/right max pooling.

out[b, c, t] = max(feat[b, c, max(t-L,0) : t+1]) + max(feat[b, c, t : min(t+R,T)])
with L = max(0, round(reg[b,t,0])), R = clip(round(reg[b,t,1]), 1, T).

Strategy (2 batches/core, pure data parallel over 8 cores, fp16 on device):
  - sparse table built in c-major layout [c%128, lev, cb, t] on the DVE:
    one full-width tensor_tensor(max) per level against a flat shifted view
    (reads that run past a level slab land in never-queried entries, so no
    pads or memsets are needed).
  - each level is transposed to t-major [t%128, tt, c] on the otherwise-idle
    PE (16 identity-matmul transposes per level into fp16 PSUM), copied
    PSUM->SBUF on the ACT engine, and stored to a per-batch DRAM row table
    [NLEV*T, C]. Level 0 rows come from a host-transposed featT input via a
    DRAM->DRAM copy.
  - the 4 RMQ terms per token (left a/b, right a/b) are fetched with 16
    hardware indirect-DMA row gathers (offsets [128, 1] int32, one 2KB row
    per partition), landing t-major in SBUF.
  - combine on DVE: max(la, lb) + max(ra, rb); store out t-major; host
    transposes [B, T, C] -> [B, C, T].
"""

import sys
import types

import numpy as np


def _install_profile_shim():
    if "antenv.axon_hooks" in sys.modules:
        return
    try:
        hooks = types.ModuleType("antenv.axon_hooks")
        hooks._hook = None
        hooks.set_axon_ntff_profile_hook = lambda h: setattr(hooks, "_hook", h)
        hooks.get_axon_ntff_profile_hook = lambda: hooks._hook
        sys.modules["antenv.axon_hooks"] = hooks
        import antenv

        antenv.axon_hooks = hooks
        from trn_agent_boot.trn_boot import _ntff_profile_via_ctypes

        hooks.set_axon_ntff_profile_hook(
            _ntff_profile_via_ctypes("/opt/axon/libaxon_pjrt.so")
        )
    except Exception:
        pass


_install_profile_shim()

import concourse.bacc as bacc
import concourse.bass as bass
import concourse.mybir as mybir
from concourse.bass_utils import run_bass_kernel_spmd

B, C, T = 16, 1024, 256
N_CORES = 8
BPC = B // N_CORES
NLEV = 6   # sparse-table levels 0..5 (windows up to 33)
NG = 8     # gathers per batch: 4 terms x 2 token chunks
CB = C // 128
LW = CB * T  # free elems per level slab per partition (2048)

_LOG2 = np.zeros(65, dtype=np.int64)
for _n in range(1, 65):
    _LOG2[_n] = _n.bit_length() - 1

_CACHE = {}
LAST_RESULT = None


def _build_graph():
    if "nc" in _CACHE:
        return _CACHE["nc"]

    nc = bacc.Bacc("TRN2", target_bir_lowering=False, debug=False,
                   num_devices=N_CORES)
    f16 = mybir.dt.float16
    i32 = mybir.dt.int32

    feat_ext = nc.dram_tensor("feat16", [BPC, C, T], f16,
                              kind="ExternalInput").ap()
    featT_ext = nc.dram_tensor("featT", [BPC, T, C], f16,
                               kind="ExternalInput").ap()
    offs_ext = nc.dram_tensor("offs", [128, BPC * NG], i32,
                              kind="ExternalInput").ap()
    out_ext = nc.dram_tensor("out", [BPC, T, C], f16,
                             kind="ExternalOutput").ap()

    tbl = [nc.dram_tensor(f"tbl{b}", [NLEV * T, C], f16).ap()
           for b in range(BPC)]

    # c-major table: flat free layout [lev][cb][t]
    cbuf = [nc.alloc_sbuf_tensor(f"cbuf{b}", [128, NLEV * LW], f16).ap()
            for b in range(BPC)]
    # t-major staging (2-level ping-pong): [slot, tt, c]
    tbuf = [nc.alloc_sbuf_tensor(f"tbuf{b}", [128, 2, 2, C], f16).ap()
            for b in range(BPC)]
    gout = [nc.alloc_sbuf_tensor(f"gout{b}", [128, 4, 2, C], f16).ap()
            for b in range(BPC)]
    obuf = [nc.alloc_sbuf_tensor(f"obuf{b}", [128, 2, C], f16).ap()
            for b in range(BPC)]
    offs_sb = nc.alloc_sbuf_tensor("offs_sb", [128, BPC * NG], i32).ap()
    ident = nc.alloc_sbuf_tensor("ident", [128, 128], f16).ap()
    rb_sb = [nc.alloc_sbuf_tensor(f"rb_sb{b}", [NLEV, 64], f16).ap()
             for b in range(BPC)]

    pbuf = [[nc.alloc_psum_tensor(f"pbuf{b}_{j}", [128, 2, C], f16).ap()
             for j in range(2)] for b in range(BPC)]

    with nc.Block() as block:
        s_inc = [nc.alloc_semaphore(f"s_inc{b}") for b in range(BPC)]
        s_ino = nc.alloc_semaphore("s_ino")
        s_id = nc.alloc_semaphore("s_id")
        s_bld = [nc.alloc_semaphore(f"s_bld{b}") for b in range(BPC)]
        s_pe = [nc.alloc_semaphore(f"s_pe{b}") for b in range(BPC)]
        s_cp = [nc.alloc_semaphore(f"s_cp{b}") for b in range(BPC)]
        s_st = [nc.alloc_semaphore(f"s_st{b}") for b in range(BPC)]
        # per-parity tbuf-slot store completion (slot = level % 2)
        s_stp = [[nc.alloc_semaphore(f"s_stp{b}_{j}") for j in range(2)]
                 for b in range(BPC)]
        s_rb = [nc.alloc_semaphore(f"s_rb{b}") for b in range(BPC)]
        s_g = [nc.alloc_semaphore(f"s_g{b}") for b in range(BPC)]
        s_cmb = [nc.alloc_semaphore(f"s_cmb{b}") for b in range(BPC)]
        s_out = [nc.alloc_semaphore(f"s_out{b}") for b in range(BPC)]

        @block.sync
        def _(sync):
            for b in range(BPC):
                sync.dma_start(
                    out=cbuf[b][:, 0:LW].rearrange(
                        "p (cb t) -> p cb t", cb=CB),
                    in_=feat_ext[b].rearrange("(cb p) t -> p cb t", p=128),
                ).then_inc(s_inc[b], 16)
            sync.dma_start(out=offs_sb, in_=offs_ext).then_inc(s_ino, 16)
            for b in range(BPC):
                # level-0 rows straight from host featT (DRAM -> DRAM)
                sync.dma_start(out=tbl[b][0:T, :],
                               in_=featT_ext[b]).then_inc(s_st[b], 16)
            for k in range(1, NLEV):
                for b in range(BPC):
                    sync.wait_ge(s_cp[b], k)
                    sync.dma_start(
                        out=tbl[b][k * T:(k + 1) * T].rearrange(
                            "(tt p) c -> p tt c", p=128),
                        in_=tbuf[b][:, k % 2, :, :],
                    ).then_inc(s_stp[b][k % 2], 16)
            for b in range(BPC):
                # all 6 level stores complete (counts are order-insensitive)
                sync.wait_ge(s_st[b], 16)
                sync.wait_ge(s_stp[b][0], 32)
                sync.wait_ge(s_stp[b][1], 48)
                # DRAM readback barrier: touch one chunk of every level's
                # rows before the indirect gathers read the table
                sync.dma_start(
                    out=rb_sb[b],
                    in_=tbl[b].rearrange("(l t) c -> l t c", t=T)[:, 0, 0:64],
                ).then_inc(s_rb[b], 16)
            for b in range(BPC):
                sync.wait_ge(s_cmb[b], 3)
                sync.dma_start(
                    out=out_ext[b].rearrange("(tt p) c -> p tt c", p=128),
                    in_=obuf[b][:, :, :],
                ).then_inc(s_out[b], 16)
            for b in range(BPC):
                sync.wait_ge(s_out[b], 16)

        @block.vector
        def _(vector):
            for k in range(1, NLEV):
                s = 1 << (k - 1)
                for b in range(BPC):
                    if k == 1:
                        vector.wait_ge(s_inc[b], 16)
                    o = (k - 1) * LW
                    vector.tensor_tensor(
                        out=cbuf[b][:, k * LW:(k + 1) * LW],
                        in0=cbuf[b][:, o:o + LW],
                        in1=cbuf[b][:, o + s:o + s + LW],
                        op=mybir.AluOpType.max,
                    ).then_inc(s_bld[b], 1)
            for b in range(BPC):
                # all 8 gathers (completion order across DMA engines is
                # arbitrary, so gate everything on the full count)
                vector.wait_ge(s_g[b], 128)
                vector.tensor_tensor(
                    out=gout[b][:, 0, :, :],
                    in0=gout[b][:, 0, :, :],
                    in1=gout[b][:, 1, :, :],
                    op=mybir.AluOpType.max,
                ).then_inc(s_cmb[b], 1)
                vector.tensor_tensor(
                    out=gout[b][:, 2, :, :],
                    in0=gout[b][:, 2, :, :],
                    in1=gout[b][:, 3, :, :],
                    op=mybir.AluOpType.max,
                ).then_inc(s_cmb[b], 1)
                vector.tensor_tensor(
                    out=obuf[b][:, :, :],
                    in0=gout[b][:, 0, :, :],
                    in1=gout[b][:, 2, :, :],
                    op=mybir.AluOpType.add,
                ).then_inc(s_cmb[b], 1)

        @block.tensor
        def _(tensor):
            tensor.wait_ge(s_id, 2)
            for k in range(1, NLEV):
                for b in range(BPC):
                    tensor.wait_ge(s_bld[b], k)
                    if k >= 3:
                        tensor.wait_ge(s_cp[b], k - 2)
                    for tt in range(2):
                        for cb in range(CB):
                            off = k * LW + cb * T + tt * 128
                            tensor.transpose(
                                out=pbuf[b][k % 2][:, tt, cb * 128:
                                                   (cb + 1) * 128],
                                in_=cbuf[b][:, off:off + 128],
                                identity=ident,
                            ).then_inc(s_pe[b], 1)

        @block.scalar
        def _(scalar):
            for k in range(1, NLEV):
                for b in range(BPC):
                    scalar.wait_ge(s_pe[b], 16 * k)
                    if k >= 3:
                        # the slot's previous store (level k-2) must be done;
                        # parity counts are order-insensitive
                        scalar.wait_ge(s_stp[b][k % 2], 16 * ((k - 1) // 2))
                    scalar.copy(
                        out=tbuf[b][:, k % 2, :, :],
                        in_=pbuf[b][k % 2][:, :, :],
                    ).then_inc(s_cp[b], 1)

        @block.gpsimd
        def _(gpsimd):
            gpsimd.memset(ident, 0.0).then_inc(s_id, 1)
            gpsimd.affine_select(
                out=ident,
                in_=ident,
                compare_op=mybir.AluOpType.not_equal,
                fill=1.0,
                base=0,
                pattern=[[-1, 128]],
                channel_multiplier=1,
            ).then_inc(s_id, 1)
            gpsimd.wait_ge(s_ino, 16)
            for b in range(BPC):
                gpsimd.wait_ge(s_rb[b], 16)
                for g in range(NG):
                    gpsimd.indirect_dma_start(
                        out=gout[b][:, g // 2, g % 2, :],
                        out_offset=None,
                        in_=tbl[b],
                        in_offset=bass.IndirectOffsetOnAxis(
                            ap=offs_sb[:, b * NG + g:b * NG + g + 1], axis=0),
                    ).then_inc(s_g[b], 16)

    nc.compile()
    _CACHE["nc"] = nc
    return nc


def _host_rows(reg):
    """Table row indices [B, 4, T] for terms (la, lb, ra, rb);
    row(level, x) = level * T + x."""
    t = np.arange(T, dtype=np.int64)[None, :]

    rl = np.maximum(np.round(reg[:, :, 0]).astype(np.int64), 0)
    l_left = np.maximum(t - rl, 0)
    len_l = t + 1 - l_left
    k_l = np.where(len_l <= 64, _LOG2[np.minimum(len_l, 64)],
                   np.floor(np.log2(len_l)).astype(np.int64))
    p_l = (1 << k_l).astype(np.int64)
    la = k_l * T + l_left
    lb = k_l * T + (t + 1 - p_l)

    rr = np.clip(np.round(reg[:, :, 1]).astype(np.int64), 1, T)
    r_right = np.minimum(t + rr, T)
    len_r = r_right - t
    k_r = np.where(len_r <= 64, _LOG2[np.minimum(len_r, 64)],
                   np.floor(np.log2(len_r)).astype(np.int64))
    p_r = (1 << k_r).astype(np.int64)
    ra = k_r * T + (t + np.zeros_like(rr))
    rb = k_r * T + (r_right - p_r)

    rows = np.stack([la, lb, ra, rb], axis=1)  # [B, 4, T]
    assert rows.min() >= 0 and rows.max() < NLEV * T, (rows.min(), rows.max())
    return rows


def kernel(feat: np.ndarray, reg: np.ndarray) -> np.ndarray:
    global LAST_RESULT
    feat = np.ascontiguousarray(feat, dtype=np.float32)
    reg = np.ascontiguousarray(reg, dtype=np.float32)
    assert feat.shape == (B, C, T) and reg.shape == (B, T, 2)

    feat16 = feat.astype(np.float16)
    featT = np.ascontiguousarray(feat16.transpose(0, 2, 1))  # [B, T, C]
    rows = _host_rows(reg)  # [B, 4, T]
    # offs[b][p, g] with g = term*2 + tt covering token t = tt*128 + p
    offs = rows.reshape(B, 4, 2, 128).reshape(B, 8, 128).astype(np.int32)
    offs = np.ascontiguousarray(offs.transpose(0, 2, 1))  # [B, 128, 8]

    nc = _build_graph()
    in_maps = []
    for i in range(N_CORES):
        sl = slice(i * BPC, (i + 1) * BPC)
        in_maps.append({
            "feat16": np.ascontiguousarray(feat16[sl]),
            "featT": np.ascontiguousarray(featT[sl]),
            "offs": np.ascontiguousarray(
                offs[sl].transpose(1, 0, 2).reshape(128, BPC * NG)),
        })

    res = run_bass_kernel_spmd(nc, in_maps, list(range(N_CORES)))
    LAST_RESULT = res
    outT = np.concatenate([res.results[i]["out"] for i in range(N_CORES)],
                          axis=0)  # [B, T, C] fp16
    return np.ascontiguousarray(outT.transpose(0, 2, 1)).astype(np.float32)
